# revision 10
# baseline (speedup 1.0000x reference)
"""Trainium2 Bass kernel for DepthwiseXCorr (SiamRPN-style head).

Pipeline per batch sample:
  k = BN+ReLU(conv1x1(kernel, w_k))      [256, 7, 7]
  s = BN+ReLU(conv1x1(search, w_s))      [256, 31, 31]
  feat = depthwise_xcorr(s, k)           [256, 25, 25]
  y = BN+ReLU(conv1x1(feat, w_h1))
  out = conv1x1(y, w_h2) + bias          [20, 25, 25]

Sharding: pure data-parallel, batch 128 -> 16 per core across 8 cores.

Implementation notes (fp8 DoubleRow design):
- Dense convs (search branch, head conv1) run as fp8 hi/lo DoubleRow
  matmuls with K=256 packed as 2 k-tiles: conv = W8h@x8h + W8h@x8l +
  W8l@x8h, three DR matmuls at 0.5 cyc/row -> 0.75x bf16 cost.  W is
  pre-scaled x16 (folded back via the BN scale) so its e4m3 hi plane
  stays in the normal range; lo planes use e5m2 for exponent range.
- The depthwise xcorr runs NA taps on the PE as fp8 DoubleRow diagonal
  matmuls, two taps per instruction: weights [c, 2, m] hold two scaled
  identities, the moving AP [c, 2, i, j] holds the two shifted search
  windows (custom-stride AP).  Each pair issues twice: against the e4m3
  hi image s8h and (same weights) against the e5m2 residual s8l, so s
  is captured to ~0.45% while k stays plain e4m3.  ~0.52 cyc/row per
  tap total, 4x faster than bf16 diag matmuls.  The remaining 49-NA
  taps run on DVE in bf16 (tensor_scalar mult + add chain folded into
  PSUM by one identity matmul).
- Diag weight tiles are built as uint16 pairs (fp8 byte in a
  parity-selected lane) with one broadcast tensor_tensor multiply
  against a constant diagonal u16 mask per (batch, chunk) on the Pool
  engine, then bitcast to fp8 for the PE weight reads.
- s8l / feat8l residuals on Pool (tensor_tensor subtract), epilogues
  (BN+ReLU + converts) on ScalarE, fp32 PSUM accumulation everywhere.
"""

import os
import sys

if "/opt/trn_rl_repo" not in sys.path:
    sys.path.insert(0, "/opt/trn_rl_repo")

import ml_dtypes
import numpy as np

B, CIN, HID, COUT = 128, 256, 256, 20
NCORES = 8
NB = B // NCORES          # batches per core
HS = 31                   # search spatial
HK = 7                    # kernel spatial
HO = HS - HK + 1          # 25, xcorr output spatial
EPS = 1e-5
GB = 4                    # batch group size for the search-branch pipeline
NCH = 2                   # channel chunks of 128
NA = int(os.environ.get("K_NA", "38"))  # xcorr taps on PE as fp8-DR
NPAIR = (NA + 1) // 2     # DR tap-pair matmuls (odd tap padded w/ zero slot)
NSLOT = max(2 * NPAIR, 2)  # diag stack slots
CONV_DR = os.environ.get("K_CONV_DR", "0") == "1"
SDR = os.environ.get("K_SDR", "1" if CONV_DR else "0") == "1"
H1DR = os.environ.get("K_H1DR", "1" if CONV_DR else "0") == "1"
NPOOL = int(os.environ.get("K_NPOOL", "2"))  # bf16 taps on Pool engine
DRI = os.environ.get("K_DRI", "0") == "1"    # SwInterleave conv weights
BF16 = ml_dtypes.bfloat16
E4M3 = ml_dtypes.float8_e4m3
E5M2 = ml_dtypes.float8_e5m2
WSCALE = 16.0             # conv weight pre-scale, folded into BN scale

_CACHE = {}


def _pair_ap(AP, img_ap, bl, ua, va, ub, vb, r0, nr):
    """Moving AP [128, 2, nr, HO] over two shifted windows of img
    [128, GB, HS, HS]: slot 0 = window (ua, va), slot 1 = (ub, vb),
    rows r0:r0+nr of the xcorr output."""
    w0 = img_ap[:, bl, ua + r0:ua + r0 + nr, va:va + HO]
    d = (ub - ua) * HS + (vb - va)
    lay = [list(w0.ap[0]), [d, 2], list(w0.ap[1]), list(w0.ap[2])]
    return AP(w0.tensor, w0.offset, lay)


def _build_nc(repeat=1):
    import concourse.bacc as bacc
    import concourse.tile as tile
    from concourse import mybir
    from concourse.bass import AP

    f32 = mybir.dt.float32
    bf16 = mybir.dt.bfloat16
    fp8 = mybir.dt.float8e4
    fp8l = mybir.dt.float8e5
    u8 = mybir.dt.uint8
    u32 = mybir.dt.uint32
    DRM = mybir.MatmulPerfMode.DoubleRow

    nc = bacc.Bacc()

    xk = nc.dram_tensor("xk", [NB, CIN, HK, HK], bf16, kind="ExternalInput")
    NGRP_ = NB // GB
    xs8h = nc.dram_tensor("xs8h", [NGRP_, 128, GB, NCH, HS * HS], fp8,
                          kind="ExternalInput")
    xs8l = nc.dram_tensor("xs8l", [NGRP_, 128, GB, NCH, HS * HS], fp8l,
                          kind="ExternalInput")
    wkT = nc.dram_tensor("wkT", [CIN, HID], bf16, kind="ExternalInput")
    wsT = nc.dram_tensor("wsT", [CIN, HID], bf16, kind="ExternalInput")
    wh1T = nc.dram_tensor("wh1T", [CIN, HID], bf16, kind="ExternalInput")
    xsbf = nc.dram_tensor("xsbf", [NB // GB, 128, GB, NCH, HS * HS], bf16,
                          kind="ExternalInput")
    ws8h = nc.dram_tensor("ws8h", [NCH, 128, HID], fp8, kind="ExternalInput")
    ws8l = nc.dram_tensor("ws8l", [NCH, 128, HID], fp8, kind="ExternalInput")
    wh18h = nc.dram_tensor("wh18h", [NCH, 128, HID], fp8, kind="ExternalInput")
    wh18l = nc.dram_tensor("wh18l", [NCH, 128, HID], fp8,
                           kind="ExternalInput")
    wh2T = nc.dram_tensor("wh2T", [HID, COUT], bf16, kind="ExternalInput")
    sck = nc.dram_tensor("sck", [HID, 1], f32, kind="ExternalInput")
    shk = nc.dram_tensor("shk", [HID, 1], f32, kind="ExternalInput")
    scs = nc.dram_tensor("scs", [HID, 1], f32, kind="ExternalInput")
    shs = nc.dram_tensor("shs", [HID, 1], f32, kind="ExternalInput")
    sch = nc.dram_tensor("sch", [HID, 1], f32, kind="ExternalInput")
    shh = nc.dram_tensor("shh", [HID, 1], f32, kind="ExternalInput")
    bh2 = nc.dram_tensor("bh2", [COUT, 1], f32, kind="ExternalInput")
    ident = nc.dram_tensor("ident", [128, 128], bf16, kind="ExternalInput")
    mask32 = nc.dram_tensor("mask32", [128, 32], u32, kind="ExternalInput")
    out = nc.dram_tensor("out", [NB, COUT, HO, HO], f32, kind="ExternalOutput")

    relu = mybir.ActivationFunctionType.Relu
    idfn = mybir.ActivationFunctionType.Identity
    copyfn = mybir.ActivationFunctionType.Copy

    TAPS = [(u, v) for u in range(HK) for v in range(HK)]

    with tile.TileContext(nc) as tc:
        with (
            tc.tile_pool(name="const", bufs=1) as cpool,
            tc.tile_pool(name="act", bufs=1) as apool,
            tc.tile_pool(name="stream", bufs=2) as spool,
            tc.tile_pool(name="diagp", bufs=3) as dpool,
            tc.tile_pool(name="psum", bufs=1, space="PSUM") as ppool,
        ):
            # ---- constants -------------------------------------------------
            wk_t, wh2_t = [], []
            sck_t, shk_t, scs_t, shs_t, sch_t, shh_t = [], [], [], [], [], []

            def _vec(vec_d, lst, nm, kc, sl):
                v = cpool.tile([128, 1], f32, name=f"{nm}_{kc}")
                nc.sync.dma_start(v[:], vec_d[sl, :])
                lst.append(v)

            for kc in range(NCH):
                sl = slice(kc * 128, (kc + 1) * 128)
                w1 = cpool.tile([128, HID], bf16, name=f"wk_{kc}")
                nc.sync.dma_start(w1[:], wkT[sl, :])
                wk_t.append(w1)
                _vec(sck, sck_t, "sck", kc, sl)
                _vec(shk, shk_t, "shk", kc, sl)
            id_t = cpool.tile([128, 128], bf16, name="id_t")
            nc.sync.dma_start(id_t[:], ident[:])
            mk_t = cpool.tile([128, 32], u32, name="mk_t")
            nc.sync.dma_start(mk_t[:], mask32[:])

            xk_ap = xk[:].rearrange("b c h w -> c b (h w)")
            xk_t = []
            for kc in range(NCH):
                t = apool.tile([128, NB, HK * HK], bf16, name=f"xk_t{kc}")
                nc.gpsimd.dma_start(t[:], xk_ap[kc * 128:(kc + 1) * 128])
                xk_t.append(t)

            # per-output-chunk contiguous DR weight tiles [c, 2, 128]
            ws8h_t, ws8l_t = [], []
            for mq in range(NCH if SDR else 0):
                msl = slice(mq * 128, (mq + 1) * 128)
                wh = cpool.tile([128, NCH, 128], fp8, name=f"ws8h_{mq}")
                nc.sync.dma_start(wh[:],
                                  ws8h[:, :, msl].rearrange("t c m -> c t m"))
                ws8h_t.append(wh)
                wl = cpool.tile([128, NCH, 128], fp8, name=f"ws8l_{mq}")
                nc.sync.dma_start(wl[:],
                                  ws8l[:, :, msl].rearrange("t c m -> c t m"))
                ws8l_t.append(wl)
            ws_t, wh1_t = [], []
            if not (SDR and H1DR):
                for kc in range(NCH):
                    sl = slice(kc * 128, (kc + 1) * 128)
                    w2 = cpool.tile([128, HID], bf16, name=f"ws_{kc}")
                    nc.sync.dma_start(w2[:], wsT[sl, :])
                    ws_t.append(w2)
                    w3 = cpool.tile([128, HID], bf16, name=f"wh1_{kc}")
                    nc.sync.dma_start(w3[:], wh1T[sl, :])
                    wh1_t.append(w3)
            for kc in range(NCH):
                sl = slice(kc * 128, (kc + 1) * 128)
                _vec(scs, scs_t, "scs", kc, sl)
                _vec(shs, shs_t, "shs", kc, sl)
            wh18h_t, wh18l_t = [], []
            for mq in range(NCH if H1DR else 0):
                msl = slice(mq * 128, (mq + 1) * 128)
                wh = cpool.tile([128, NCH, 128], fp8, name=f"wh18h_{mq}")
                nc.sync.dma_start(wh[:],
                                  wh18h[:, :, msl].rearrange("t c m -> c t m"))
                wh18h_t.append(wh)
                wl = cpool.tile([128, NCH, 128], fp8, name=f"wh18l_{mq}")
                nc.sync.dma_start(wl[:],
                                  wh18l[:, :, msl].rearrange("t c m -> c t m"))
                wh18l_t.append(wl)
            for kc in range(NCH):
                sl = slice(kc * 128, (kc + 1) * 128)
                w4 = cpool.tile([128, COUT], bf16, name=f"wh2_{kc}")
                nc.sync.dma_start(w4[:], wh2T[sl, :])
                wh2_t.append(w4)
                _vec(sch, sch_t, "sch", kc, sl)
                _vec(shh, shh_t, "shh", kc, sl)
            bh2_t = cpool.tile([COUT, 1], f32, name="bh2_t")
            nc.sync.dma_start(bh2_t[:], bh2[:])

            # ---- kernel branch conv (all NB batches at once) ---------------
            k_feat = []
            for mc in range(NCH):
                kf = apool.tile([128, NB, HK * HK], f32, name=f"k_feat{mc}")
                for half in range(2):
                    bs = slice(half * (NB // 2), (half + 1) * (NB // 2))
                    ps = ppool.tile([128, NB // 2, HK * HK], f32,
                                    name="ps_cs", tag="csA", bufs=1)
                    for kc in range(NCH):
                        nc.tensor.matmul(
                            ps[:],
                            wk_t[kc][:, mc * 128:(mc + 1) * 128],
                            xk_t[kc][:, bs, :],
                            start=(kc == 0), stop=(kc == NCH - 1),
                        )
                    nc.scalar.activation(kf[:, bs, :], ps[:], relu,
                                         bias=shk_t[mc][:], scale=sck_t[mc][:])
                k_feat.append(kf)

            # ---- k-prep: fp8 bytes replicated into all 4 u32 lanes ---------
            # krep[mc][c, b, slot] (u32 view): fp8e4(k) byte x4; slot
            # NSLOT-1 stays zero (memset) when NA is odd.
            krep_t = []
            for mc in range(NCH):
                if not NA:
                    break
                k8 = apool.tile([128, NB * HK * HK], fp8, name=f"k8_{mc}")
                nc.scalar.activation(
                    k8[:], k_feat[mc][:].rearrange("c b f -> c (b f)"), copyfn)
                kr = apool.tile([128, NB, NSLOT, 4], u8, name=f"kr_{mc}")
                if NA < NSLOT:
                    nc.vector.memset(kr[:], 0)
                nc.vector.tensor_copy(
                    kr[:, :, 0:NA, :],
                    k8[:].bitcast(u8)
                        .rearrange("c (b f) -> c b f", b=NB)[:, :, 0:NA]
                        .unsqueeze(3).broadcast_to([128, NB, NA, 4]))
                krep_t.append(kr)

            for _rep in range(repeat):
              # ---- main loop over batch groups -----------------------------
              NGRP = NB // GB
              ys = {}

              def alloc_group(g):
                  """DMA loads + tile allocation for one batch group; conv
                  compute is emitted piecewise via emit_conv_piece."""
                  tiles = {}
                  if SDR:
                      xh = spool.tile([128, GB, NCH, HS * HS], fp8,
                                      name="xh", tag="xh")
                      nc.gpsimd.dma_start(xh[:], xs8h[g][:])
                      xl = spool.tile([128, GB, NCH, HS * HS], fp8l,
                                      name="xl", tag="xl")
                      nc.gpsimd.dma_start(xl[:], xs8l[g][:])
                      tiles["x"] = (xh, xl)
                  else:
                      xb = spool.tile([128, GB, NCH, HS * HS], bf16,
                                      name="xb", tag="xh")
                      nc.gpsimd.dma_start(xb[:], xsbf[g][:])
                      tiles["x"] = (xb,)
                  for mc in range(NCH):
                      s8h = spool.tile([128, GB, HS, HS], fp8,
                                       name=f"s8h{mc}", tag=f"s8h{mc}")
                      s8l = spool.tile([128, GB, HS, HS], fp8l,
                                       name=f"s8l{mc}", tag=f"s8l{mc}")
                      sbf = spool.tile([128, GB, HS, HS], bf16,
                                       name=f"sbf{mc}", tag=f"sbf{mc}")
                      tiles[mc] = (s8h, s8l, sbf)
                  return tiles

              def emit_conv_piece(tiles, bl, mc):
                  """Search conv + epilogues + residual for one (bl, mc).
                  Weight-outer / phase-inner so each conv LDWEIGHTS serves
                  both PSUM phases back-to-back."""
                  s8h, s8l, sbf = tiles[mc]
                  msl = slice(mc * 128, (mc + 1) * 128)
                  for ph, (r0, r1) in enumerate(((0, 16), (16, 31))):
                      ps = ppool.tile([128, r1 - r0, HS], f32,
                                      name="ps_cs2",
                                      tag=("csA" if ph == 0 else "csB"),
                                      bufs=1)
                      fsl = slice(r0 * HS, r1 * HS)
                      if SDR:
                          CDRM = (mybir.MatmulPerfMode.DoubleRowSwInterleave
                                  if DRI else DRM)
                          xh, xl = tiles["x"]
                          nc.tensor.matmul(
                              ps[:], ws8h_t[mc][:],
                              xh[:, bl, :, fsl], start=True,
                              stop=False, perf_mode=CDRM)
                          nc.tensor.matmul(
                              ps[:], ws8h_t[mc][:],
                              xl[:, bl, :, fsl], start=False,
                              stop=False, perf_mode=CDRM)
                          nc.tensor.matmul(
                              ps[:], ws8l_t[mc][:],
                              xh[:, bl, :, fsl], start=False,
                              stop=True, perf_mode=CDRM)
                      else:
                          (xb,) = tiles["x"]
                          for kc in range(NCH):
                              nc.tensor.matmul(
                                  ps[:], ws_t[kc][:, msl],
                                  xb[:, bl, kc, fsl],
                                  start=(kc == 0),
                                  stop=(kc == NCH - 1))
                      nc.scalar.activation(s8h[:, bl, r0:r1, :], ps[:],
                                           relu, bias=shs_t[mc][:],
                                           scale=scs_t[mc][:])
                      nc.scalar.activation(sbf[:, bl, r0:r1, :], ps[:],
                                           relu, bias=shs_t[mc][:],
                                           scale=scs_t[mc][:])
                  nc.gpsimd.tensor_tensor(
                      s8l[:, bl], sbf[:, bl], s8h[:, bl],
                      mybir.AluOpType.subtract)

              def emit_group_conv(g):
                  tiles = alloc_group(g)
                  for mc in range(NCH):
                      for bl in range(GB):
                          emit_conv_piece(tiles, bl, mc)
                  return [tiles[mc] for mc in range(NCH)]

              s_feat = emit_group_conv(0)
              tiles_next = None
              piece_idx = 0
              def emit_stacks(b_abs):
                  """Diag stack builds (DVE u32 AND) for both chunks of one
                  batch; hoisted a batch ahead so the PE never waits."""
                  if not NA:
                      return None
                  res = []
                  for mc in range(NCH):
                      st = dpool.tile([128, NSLOT, 32], u32,
                                      name="st", tag=f"st{mc}")
                      nc.vector.tensor_tensor(
                          st[:],
                          krep_t[mc][:, b_abs].bitcast(u32)
                              .broadcast_to([128, NSLOT, 32]),
                          mk_t[:].unsqueeze(1)
                              .broadcast_to([128, NSLOT, 32]),
                          mybir.AluOpType.bitwise_and)
                      res.append(st[:].bitcast(fp8))  # [128, NSLOT, 128]
                  return res

              stacks = emit_stacks(0)
              for g in range(NGRP):
                  s_feat_next = None
                  if g + 1 < NGRP:
                      tiles_next = alloc_group(g + 1)
                      piece_idx = 0
                      s_feat_next = [tiles_next[mc] for mc in range(NCH)]
                  for bl in range(GB):
                      b_abs = g * GB + bl
                      cur_stacks = stacks
                      if b_abs + 1 < NB:
                          stacks = emit_stacks(b_abs + 1)
                      # feat tiles hold both channel chunks: [128, 2, 625]
                      f8h = f8l = None
                      if H1DR:
                          f8h = apool.tile([128, NCH, HO * HO], fp8,
                                           name="f8h", tag="f8h", bufs=2)
                          f8l = apool.tile([128, NCH, HO * HO], fp8l,
                                           name="f8l", tag="f8l", bufs=2)
                      fbf = apool.tile([128, NCH, HO * HO], bf16,
                                       name="fbf", tag="fbf", bufs=2)
                      for mc in range(NCH):
                          s8h, s8l, sbf = s_feat[mc]
                          if NA:
                              st8 = cur_stacks[mc]

                          # bf16 taps NA..48: NPOOL of them on Pool (own
                          # accumulator, merged on Pool), the rest on DVE
                          kcol = lambda uv: k_feat[mc][:, b_abs, uv:uv + 1]
                          n_rest = 49 - NA
                          n_pool = min(NPOOL, max(0, n_rest - 1))
                          n_dve = n_rest - n_pool
                          accd = None
                          if n_rest:
                              accd = apool.tile([128, HO, HO], bf16,
                                                name="accd", tag=f"accd{mc}",
                                                bufs=4)
                              for i in range(n_dve):
                                  u, v = TAPS[NA + i]
                                  win = sbf[:, bl, u:u + HO, v:v + HO]
                                  if i == 0:
                                      nc.vector.tensor_scalar_mul(
                                          accd[:], win, kcol(NA + i))
                                  else:
                                      tmp = apool.tile([128, HO, HO], bf16,
                                                       name="tmp", tag="tmp",
                                                       bufs=6)
                                      nc.vector.tensor_scalar_mul(
                                          tmp[:], win, kcol(NA + i))
                                      nc.vector.tensor_tensor(
                                          accd[:], accd[:], tmp[:],
                                          mybir.AluOpType.add)
                              accp = None
                              for i in range(n_pool):
                                  uv = NA + n_dve + i
                                  u, v = TAPS[uv]
                                  win = sbf[:, bl, u:u + HO, v:v + HO]
                                  if i == 0:
                                      accp = apool.tile([128, HO, HO], bf16,
                                                        name="accp",
                                                        tag=f"accp{mc}",
                                                        bufs=4)
                                      nc.gpsimd.tensor_scalar_mul(
                                          accp[:], win, kcol(uv))
                                  else:
                                      tmp = apool.tile([128, HO, HO], bf16,
                                                       name="tmpp", tag="tmpp",
                                                       bufs=6)
                                      nc.gpsimd.tensor_scalar_mul(
                                          tmp[:], win, kcol(uv))
                                      nc.gpsimd.tensor_tensor(
                                          accp[:], accp[:], tmp[:],
                                          mybir.AluOpType.add)
                              if accp is not None:
                                  nc.gpsimd.tensor_tensor(
                                      accd[:], accd[:], accp[:],
                                      mybir.AluOpType.add)

                          # xcorr phases: psA rows 0:20, psB rows 20:25.
                          # t-outer so one diag LDWEIGHTS serves 4 matmuls
                          # (hi/lo x both phases) back-to-back.
                          PHS = ((0, 20), (20, 25))
                          pss = [ppool.tile([128, (r1 - r0) * HO], f32,
                                            name=f"ps_xc{ph}", tag=f"xc{ph}",
                                            bufs=2)
                                 for ph, (r0, r1) in enumerate(PHS)]
                          for t in range(NPAIR):
                              ua, va = TAPS[2 * t]
                              if 2 * t + 1 < NA:
                                  ub, vb = TAPS[2 * t + 1]
                              else:
                                  ub, vb = ua, va  # zero-slot pad
                              wts = st8[:, 2 * t:2 * t + 2, :]
                              for j, img in enumerate((s8h, s8l)):
                                  for ph, (r0, r1) in enumerate(PHS):
                                      rhs = _pair_ap(AP, img[:], bl, ua, va,
                                                     ub, vb, r0, r1 - r0)
                                      nc.tensor.matmul(
                                          pss[ph][:], wts, rhs,
                                          start=(t == 0 and j == 0),
                                          stop=(t == NPAIR - 1 and j == 1
                                                and accd is None),
                                          perf_mode=DRM)
                          for ph, (r0, r1) in enumerate(PHS):
                              ps = pss[ph]
                              if accd is not None:
                                  nc.tensor.matmul(
                                      ps[:], id_t[:], accd[:, r0:r1, :],
                                      start=(NPAIR == 0), stop=True)
                              c0 = r0 * HO
                              c1 = r1 * HO
                              if H1DR:
                                  nc.scalar.activation(f8h[:, mc, c0:c1],
                                                       ps[:], copyfn)
                              nc.scalar.activation(fbf[:, mc, c0:c1], ps[:],
                                                   copyfn)
                          if H1DR:
                              nc.gpsimd.tensor_tensor(
                                  f8l[:, mc], fbf[:, mc], f8h[:, mc],
                                  mybir.AluOpType.subtract)

                          # interleave one conv piece of the next group so
                          # the PE + epilogue engines stay ahead
                          if s_feat_next is not None and piece_idx < NCH * GB:
                              emit_conv_piece(tiles_next, piece_idx // NCH,
                                              piece_idx % NCH)
                              piece_idx += 1

                          if mc < NCH - 1:
                              continue

                          # head conv1 + BN/ReLU
                          for mq in range(NCH):
                              yt = apool.tile([128, HO * HO], bf16,
                                              name=f"y{mq}", tag=f"y{mq}",
                                              bufs=2)
                              qsl = slice(mq * 128, (mq + 1) * 128)
                              for ph, (c0, c1) in enumerate(((0, 500),
                                                            (500, 625))):
                                  ps = ppool.tile([128, c1 - c0], f32,
                                                  name="ps_hd",
                                                  tag=("hdA" if ph == 0
                                                       else "hdB"),
                                                  bufs=1)
                                  if H1DR:
                                      CDRM = (
                                          mybir.MatmulPerfMode
                                          .DoubleRowSwInterleave
                                          if DRI else DRM)
                                      nc.tensor.matmul(
                                          ps[:], wh18h_t[mq][:],
                                          f8h[:, :, c0:c1], start=True,
                                          stop=False, perf_mode=CDRM)
                                      nc.tensor.matmul(
                                          ps[:], wh18h_t[mq][:],
                                          f8l[:, :, c0:c1], start=False,
                                          stop=False, perf_mode=CDRM)
                                      nc.tensor.matmul(
                                          ps[:], wh18l_t[mq][:],
                                          f8h[:, :, c0:c1], start=False,
                                          stop=True, perf_mode=CDRM)
                                  else:
                                      for kc in range(NCH):
                                          nc.tensor.matmul(
                                              ps[:], wh1_t[kc][:, qsl],
                                              fbf[:, kc, c0:c1],
                                              start=(kc == 0),
                                              stop=(kc == NCH - 1))
                                  nc.scalar.activation(yt[:, c0:c1], ps[:],
                                                       relu, bias=shh_t[mq][:],
                                                       scale=sch_t[mq][:])
                              ys[mq] = yt

                          # head conv2 + bias (bf16)
                          ot = apool.tile([COUT, HO * HO], f32,
                                          name="ot", tag="ot", bufs=2)
                          for ph, (c0, c1) in enumerate(((0, 500), (500, 625))):
                              ps = ppool.tile([COUT, c1 - c0], f32,
                                              name="ps_o",
                                              tag=("hdA" if ph == 0 else "hdB"),
                                              bufs=1)
                              for kc in range(NCH):
                                  nc.tensor.matmul(
                                      ps[:],
                                      wh2_t[kc][:],
                                      ys[kc][:, c0:c1],
                                      start=(kc == 0), stop=(kc == NCH - 1),
                                  )
                              nc.scalar.activation(ot[:, c0:c1], ps[:], idfn,
                                                   bias=bh2_t[:], scale=1.0)
                          nc.sync.dma_start(
                              out[b_abs][:].rearrange("o h w -> o (h w)"),
                              ot[:])
                  if s_feat_next is not None:
                      s_feat = s_feat_next

    nc.compile()
    return nc


def _get_nc():
    if "nc" not in _CACHE:
        _CACHE["nc"] = _build_nc()
    return _CACHE["nc"]


def _host_prep(w, dt_hi, dt_lo, scale=1.0):
    ws = np.asarray(w, np.float32) * scale
    hi = ws.astype(dt_hi)
    lo = (ws - hi.astype(np.float32)).astype(dt_lo)
    return hi, lo


def build_in_maps(kernel, search, w_k, g_k, b_k, m_k, v_k, w_s, g_s, b_s,
                  m_s, v_s, w_h1, g_h, b_h, m_h, v_h, w_h2, bias_h2):
    def fold(g, b, m, v, wscale=1.0):
        sc = (np.asarray(g) / np.sqrt(np.asarray(v) + EPS) / wscale)
        sh = (np.asarray(b) - np.asarray(m) * sc * wscale)
        return (sc.astype(np.float32).reshape(-1, 1),
                sh.astype(np.float32).reshape(-1, 1))

    kernel, search, w_k, w_s, w_h1, w_h2, bias_h2 = [
        np.asarray(a) for a in
        (kernel, search, w_k, w_s, w_h1, w_h2, bias_h2)]

    global DRI
    sck, shk = fold(g_k, b_k, m_k, v_k)
    scs, shs = fold(g_s, b_s, m_s, v_s, WSCALE if SDR else 1.0)
    sch, shh = fold(g_h, b_h, m_h, v_h, WSCALE if H1DR else 1.0)

    # fp8 hi/lo conv weights, pre-scaled x16, laid out [kc, 128, HID]
    # (transposed: contraction dim on rows)
    wsT = np.ascontiguousarray(w_s.T).astype(np.float32)      # [CIN, HID]
    wh1T = np.ascontiguousarray(w_h1.T).astype(np.float32)
    ws8h, ws8l = _host_prep(wsT, E4M3, E4M3, WSCALE)
    wh18h, wh18l = _host_prep(wh1T, E4M3, E4M3, WSCALE)

    def _interleave(w):
        # [256, 256] -> per 128-out-chunk DoubleRowSwInterleave layout:
        # stored[:, 2k] = A[:, M-1-k], stored[:, 2k+1] = B[:, M-1-k]
        # where A/B are the two k-tiles; returns same [NCH, 128, HID] shape
        # consumed by the per-chunk tile loads.
        r = np.empty((NCH, 128, HID), w.dtype)
        wv = w.reshape(NCH, 128, HID)
        for mq in range(NCH):
            A = wv[0, :, mq * 128:(mq + 1) * 128]
            B = wv[1, :, mq * 128:(mq + 1) * 128]
            st = np.empty((128, 256), w.dtype)
            st[:, 0::2] = A[:, ::-1]
            st[:, 1::2] = B[:, ::-1]
            # store back into the [t, c, m] layout the DMA rearrange expects:
            # tile [c, t, m] = st[c, t + 2*?]... tile slot t, col m reads
            # stored flat index 2*m + t? No: tile AP [c, 2, 128] reads
            # flat [c, 256]; keep flat layout in slot-major order.
            r[0, :, mq * 128:(mq + 1) * 128] = st[:, 0:128]
            r[1, :, mq * 128:(mq + 1) * 128] = st[:, 128:256]
        return r

    if DRI:
        ws8h, ws8l = _interleave(ws8h.reshape(NCH, 128, HID)), _interleave(
            ws8l.reshape(NCH, 128, HID))
        wh18h, wh18l = _interleave(
            wh18h.reshape(NCH, 128, HID)), _interleave(
            wh18l.reshape(NCH, 128, HID))

    mask32 = np.zeros((128, 32), np.uint32)
    for c in range(128):
        mask32[c, c // 4] = np.uint32(0xFF) << np.uint32(8 * (c % 4))

    common = {
        "wkT": np.ascontiguousarray(w_k.T).astype(BF16),
        "wsT": wsT.astype(BF16),
        "wh1T": wh1T.astype(BF16),
        "ws8h": ws8h.reshape(NCH, 128, HID),
        "ws8l": ws8l.reshape(NCH, 128, HID),
        "wh18h": wh18h.reshape(NCH, 128, HID),
        "wh18l": wh18l.reshape(NCH, 128, HID),
        "wh2T": np.ascontiguousarray(w_h2.T).astype(BF16),
        "sck": sck, "shk": shk, "scs": scs, "shs": shs,
        "sch": sch, "shh": shh,
        "bh2": bias_h2.astype(np.float32).reshape(-1, 1),
        "ident": np.eye(128, dtype=BF16),
        "mask32": mask32,
    }
    xk_all = kernel.astype(BF16)
    xs = search.reshape(B, CIN, HS * HS).astype(np.float32)
    xs8h_all = xs.astype(E4M3)
    xs8l_all = (xs - xs8h_all.astype(np.float32)).astype(E5M2)

    def _grp(a):
        # [NB, 256, F] -> [NGRP, 128, GB, NCH, F] (tile layout per group)
        ngrp = NB // GB
        return np.ascontiguousarray(
            a.reshape(ngrp, GB, NCH, 128, HS * HS).transpose(0, 3, 1, 2, 4))

    in_maps = []
    for i in range(NCORES):
        bs = slice(i * NB, (i + 1) * NB)
        m = dict(common)
        m["xk"] = np.ascontiguousarray(xk_all[bs])
        m["xs8h"] = _grp(xs8h_all[bs])
        m["xs8l"] = _grp(xs8l_all[bs])
        m["xsbf"] = _grp(xs[bs].astype(BF16))
        in_maps.append(m)
    return in_maps


def kernel(**inputs):
    from concourse.bass_utils import run_bass_kernel_spmd

    in_maps = build_in_maps(**{k: np.asarray(v) for k, v in inputs.items()})
    nc = _get_nc()
    res = run_bass_kernel_spmd(nc, in_maps, core_ids=list(range(NCORES)))
    return np.concatenate([res.results[i]["out"] for i in range(NCORES)],
                          axis=0)



# revision 12
# speedup vs baseline: 1.0039x; 1.0039x over previous
"""Trainium2 Bass kernel for DepthwiseXCorr (SiamRPN-style head).

Pipeline per batch sample:
  k = BN+ReLU(conv1x1(kernel, w_k))      [256, 7, 7]
  s = BN+ReLU(conv1x1(search, w_s))      [256, 31, 31]
  feat = depthwise_xcorr(s, k)           [256, 25, 25]
  y = BN+ReLU(conv1x1(feat, w_h1))
  out = conv1x1(y, w_h2) + bias          [20, 25, 25]

Sharding: pure data-parallel, batch 128 -> 16 per core across 8 cores.

Implementation notes (fp8 DoubleRow design):
- Dense convs (search branch, head conv1) run as fp8 hi/lo DoubleRow
  matmuls with K=256 packed as 2 k-tiles: conv = W8h@x8h + W8h@x8l +
  W8l@x8h, three DR matmuls at 0.5 cyc/row -> 0.75x bf16 cost.  W is
  pre-scaled x16 (folded back via the BN scale) so its e4m3 hi plane
  stays in the normal range; lo planes use e5m2 for exponent range.
- The depthwise xcorr runs NA taps on the PE as fp8 DoubleRow diagonal
  matmuls, two taps per instruction: weights [c, 2, m] hold two scaled
  identities, the moving AP [c, 2, i, j] holds the two shifted search
  windows (custom-stride AP).  Each pair issues twice: against the e4m3
  hi image s8h and (same weights) against the e5m2 residual s8l, so s
  is captured to ~0.45% while k stays plain e4m3.  ~0.52 cyc/row per
  tap total, 4x faster than bf16 diag matmuls.  The remaining 49-NA
  taps run on DVE in bf16 (tensor_scalar mult + add chain folded into
  PSUM by one identity matmul).
- Diag weight tiles are built as uint16 pairs (fp8 byte in a
  parity-selected lane) with one broadcast tensor_tensor multiply
  against a constant diagonal u16 mask per (batch, chunk) on the Pool
  engine, then bitcast to fp8 for the PE weight reads.
- s8l / feat8l residuals on Pool (tensor_tensor subtract), epilogues
  (BN+ReLU + converts) on ScalarE, fp32 PSUM accumulation everywhere.
"""

import os
import sys

if "/opt/trn_rl_repo" not in sys.path:
    sys.path.insert(0, "/opt/trn_rl_repo")

import ml_dtypes
import numpy as np

B, CIN, HID, COUT = 128, 256, 256, 20
NCORES = 8
NB = B // NCORES          # batches per core
HS = 31                   # search spatial
HK = 7                    # kernel spatial
HO = HS - HK + 1          # 25, xcorr output spatial
EPS = 1e-5
GB = 4                    # batch group size for the search-branch pipeline
NCH = 2                   # channel chunks of 128
NA = int(os.environ.get("K_NA", "38"))  # xcorr taps on PE as fp8-DR
NPAIR = (NA + 1) // 2     # DR tap-pair matmuls (odd tap padded w/ zero slot)
NSLOT = max(2 * NPAIR, 2)  # diag stack slots
CONV_DR = os.environ.get("K_CONV_DR", "0") == "1"
SDR = os.environ.get("K_SDR", "1" if CONV_DR else "0") == "1"
H1DR = os.environ.get("K_H1DR", "1" if CONV_DR else "0") == "1"
NPOOL = int(os.environ.get("K_NPOOL", "3"))  # bf16 taps on Pool engine
DRI = os.environ.get("K_DRI", "0") == "1"    # SwInterleave conv weights
BF16 = ml_dtypes.bfloat16
E4M3 = ml_dtypes.float8_e4m3
E5M2 = ml_dtypes.float8_e5m2
WSCALE = 16.0             # conv weight pre-scale, folded into BN scale

_CACHE = {}


HSP = 32                  # padded row stride of s tiles (DVE 4B alignment)


def _pair_ap(AP, img_ap, bl, ua, va, ub, vb, r0, nr):
    """Moving AP [128, 2, nr, HO] over two shifted windows of img
    [128, GB, HS, HSP]: slot 0 = window (ua, va), slot 1 = (ub, vb),
    rows r0:r0+nr of the xcorr output."""
    w0 = img_ap[:, bl, ua + r0:ua + r0 + nr, va:va + HO]
    d = (ub - ua) * HSP + (vb - va)
    lay = [list(w0.ap[0]), [d, 2], list(w0.ap[1]), list(w0.ap[2])]
    return AP(w0.tensor, w0.offset, lay)


def _tap_order():
    """Tap order: with NA=38, PE gets the 21 odd-v taps + 17 even-v taps
    (rows 0..3 and (4,0)); DVE/Pool get the remaining 11 even-v taps
    (4B-aligned windows).  Otherwise row-major."""
    if NA != 38:
        return [(u, v) for u in range(HK) for v in range(HK)], False
    odd = [(u, v) for u in range(HK) for v in (1, 3, 5)]
    even_pe = [(u, v) for u in range(4) for v in (0, 2, 4, 6)] + [(4, 0)]
    rest = ([(4, v) for v in (2, 4, 6)]
            + [(u, v) for u in range(5, HK) for v in (0, 2, 4, 6)])
    return odd + even_pe + rest, True


def _build_nc(repeat=1):
    import concourse.bacc as bacc
    import concourse.tile as tile
    from concourse import mybir
    from concourse.bass import AP

    f32 = mybir.dt.float32
    bf16 = mybir.dt.bfloat16
    fp8 = mybir.dt.float8e4
    fp8l = mybir.dt.float8e5
    u8 = mybir.dt.uint8
    u32 = mybir.dt.uint32
    DRM = mybir.MatmulPerfMode.DoubleRow

    nc = bacc.Bacc()

    xk = nc.dram_tensor("xk", [NB, CIN, HK, HK], bf16, kind="ExternalInput")
    NGRP_ = NB // GB
    xs8h = nc.dram_tensor("xs8h", [NGRP_, 128, GB, NCH, HS * HS], fp8,
                          kind="ExternalInput")
    xs8l = nc.dram_tensor("xs8l", [NGRP_, 128, GB, NCH, HS * HS], fp8l,
                          kind="ExternalInput")
    wkT = nc.dram_tensor("wkT", [CIN, HID], bf16, kind="ExternalInput")
    wsT = nc.dram_tensor("wsT", [CIN, HID], bf16, kind="ExternalInput")
    wh1T = nc.dram_tensor("wh1T", [CIN, HID], bf16, kind="ExternalInput")
    xsbf = nc.dram_tensor("xsbf", [NB // GB, 128, GB, NCH, HS * HS], bf16,
                          kind="ExternalInput")
    ws8h = nc.dram_tensor("ws8h", [NCH, 128, HID], fp8, kind="ExternalInput")
    ws8l = nc.dram_tensor("ws8l", [NCH, 128, HID], fp8, kind="ExternalInput")
    wh18h = nc.dram_tensor("wh18h", [NCH, 128, HID], fp8, kind="ExternalInput")
    wh18l = nc.dram_tensor("wh18l", [NCH, 128, HID], fp8,
                           kind="ExternalInput")
    wh2T = nc.dram_tensor("wh2T", [HID, COUT], bf16, kind="ExternalInput")
    sck = nc.dram_tensor("sck", [HID, 1], f32, kind="ExternalInput")
    shk = nc.dram_tensor("shk", [HID, 1], f32, kind="ExternalInput")
    scs = nc.dram_tensor("scs", [HID, 1], f32, kind="ExternalInput")
    shs = nc.dram_tensor("shs", [HID, 1], f32, kind="ExternalInput")
    sch = nc.dram_tensor("sch", [HID, 1], f32, kind="ExternalInput")
    shh = nc.dram_tensor("shh", [HID, 1], f32, kind="ExternalInput")
    bh2 = nc.dram_tensor("bh2", [COUT, 1], f32, kind="ExternalInput")
    ident = nc.dram_tensor("ident", [128, 128], bf16, kind="ExternalInput")
    mask32 = nc.dram_tensor("mask32", [128, 32], u32, kind="ExternalInput")
    out = nc.dram_tensor("out", [NB, COUT, HO, HO], f32, kind="ExternalOutput")

    relu = mybir.ActivationFunctionType.Relu
    idfn = mybir.ActivationFunctionType.Identity
    copyfn = mybir.ActivationFunctionType.Copy

    TAPS, PERM = _tap_order()

    with tile.TileContext(nc) as tc:
        with (
            tc.tile_pool(name="const", bufs=1) as cpool,
            tc.tile_pool(name="act", bufs=1) as apool,
            tc.tile_pool(name="stream", bufs=2) as spool,
            tc.tile_pool(name="diagp", bufs=3) as dpool,
            tc.tile_pool(name="psum", bufs=1, space="PSUM") as ppool,
        ):
            # ---- constants -------------------------------------------------
            wk_t, wh2_t = [], []
            sck_t, shk_t, scs_t, shs_t, sch_t, shh_t = [], [], [], [], [], []

            def _vec(vec_d, lst, nm, kc, sl):
                v = cpool.tile([128, 1], f32, name=f"{nm}_{kc}")
                nc.sync.dma_start(v[:], vec_d[sl, :])
                lst.append(v)

            for kc in range(NCH):
                sl = slice(kc * 128, (kc + 1) * 128)
                w1 = cpool.tile([128, HID], bf16, name=f"wk_{kc}")
                nc.sync.dma_start(w1[:], wkT[sl, :])
                wk_t.append(w1)
                _vec(sck, sck_t, "sck", kc, sl)
                _vec(shk, shk_t, "shk", kc, sl)
            id_t = cpool.tile([128, 128], bf16, name="id_t")
            nc.sync.dma_start(id_t[:], ident[:])
            mk_t = cpool.tile([128, 32], u32, name="mk_t")
            nc.sync.dma_start(mk_t[:], mask32[:])

            xk_ap = xk[:].rearrange("b c h w -> c b (h w)")
            xk_t = []
            for kc in range(NCH):
                t = apool.tile([128, NB, HK * HK], bf16, name=f"xk_t{kc}")
                nc.gpsimd.dma_start(t[:], xk_ap[kc * 128:(kc + 1) * 128])
                xk_t.append(t)

            # per-output-chunk contiguous DR weight tiles [c, 2, 128]
            ws8h_t, ws8l_t = [], []
            for mq in range(NCH if SDR else 0):
                msl = slice(mq * 128, (mq + 1) * 128)
                wh = cpool.tile([128, NCH, 128], fp8, name=f"ws8h_{mq}")
                nc.sync.dma_start(wh[:],
                                  ws8h[:, :, msl].rearrange("t c m -> c t m"))
                ws8h_t.append(wh)
                wl = cpool.tile([128, NCH, 128], fp8, name=f"ws8l_{mq}")
                nc.sync.dma_start(wl[:],
                                  ws8l[:, :, msl].rearrange("t c m -> c t m"))
                ws8l_t.append(wl)
            ws_t, wh1_t = [], []
            if not (SDR and H1DR):
                for kc in range(NCH):
                    sl = slice(kc * 128, (kc + 1) * 128)
                    w2 = cpool.tile([128, HID], bf16, name=f"ws_{kc}")
                    nc.sync.dma_start(w2[:], wsT[sl, :])
                    ws_t.append(w2)
                    w3 = cpool.tile([128, HID], bf16, name=f"wh1_{kc}")
                    nc.sync.dma_start(w3[:], wh1T[sl, :])
                    wh1_t.append(w3)
            for kc in range(NCH):
                sl = slice(kc * 128, (kc + 1) * 128)
                _vec(scs, scs_t, "scs", kc, sl)
                _vec(shs, shs_t, "shs", kc, sl)
            wh18h_t, wh18l_t = [], []
            for mq in range(NCH if H1DR else 0):
                msl = slice(mq * 128, (mq + 1) * 128)
                wh = cpool.tile([128, NCH, 128], fp8, name=f"wh18h_{mq}")
                nc.sync.dma_start(wh[:],
                                  wh18h[:, :, msl].rearrange("t c m -> c t m"))
                wh18h_t.append(wh)
                wl = cpool.tile([128, NCH, 128], fp8, name=f"wh18l_{mq}")
                nc.sync.dma_start(wl[:],
                                  wh18l[:, :, msl].rearrange("t c m -> c t m"))
                wh18l_t.append(wl)
            for kc in range(NCH):
                sl = slice(kc * 128, (kc + 1) * 128)
                w4 = cpool.tile([128, COUT], bf16, name=f"wh2_{kc}")
                nc.sync.dma_start(w4[:], wh2T[sl, :])
                wh2_t.append(w4)
                _vec(sch, sch_t, "sch", kc, sl)
                _vec(shh, shh_t, "shh", kc, sl)
            bh2_t = cpool.tile([COUT, 1], f32, name="bh2_t")
            nc.sync.dma_start(bh2_t[:], bh2[:])

            # ---- kernel branch conv (all NB batches at once) ---------------
            k_feat = []
            for mc in range(NCH):
                kf = apool.tile([128, NB, HK * HK], f32, name=f"k_feat{mc}")
                for half in range(2):
                    bs = slice(half * (NB // 2), (half + 1) * (NB // 2))
                    ps = ppool.tile([128, NB // 2, HK * HK], f32,
                                    name="ps_cs", tag="csA", bufs=1)
                    for kc in range(NCH):
                        nc.tensor.matmul(
                            ps[:],
                            wk_t[kc][:, mc * 128:(mc + 1) * 128],
                            xk_t[kc][:, bs, :],
                            start=(kc == 0), stop=(kc == NCH - 1),
                        )
                    nc.scalar.activation(kf[:, bs, :], ps[:], relu,
                                         bias=shk_t[mc][:], scale=sck_t[mc][:])
                k_feat.append(kf)

            # ---- k-prep: fp8 bytes replicated into all 4 u32 lanes ---------
            # krep[mc][c, b, slot] (u32 view): fp8e4(k) byte x4; slot
            # NSLOT-1 stays zero (memset) when NA is odd.
            krep_t = []
            for mc in range(NCH):
                if not NA:
                    break
                k8 = apool.tile([128, NB * HK * HK], fp8, name=f"k8_{mc}")
                if PERM:
                    # permuted tap order: slots 0:21 odd-v, 21:37 even-v
                    # rows 0..3, slot 37 = (4,0) (strided gathers from
                    # row-major k_feat)
                    kfv = k_feat[mc][:].rearrange("c b (u v) -> c b u v",
                                                  u=HK)
                    k8v = k8[:].rearrange("c (b f) -> c b f", b=NB)
                    nc.scalar.activation(
                        k8v[:, :, 0:21].rearrange("c b (u w) -> c b u w",
                                                  u=HK),
                        kfv[:, :, :, 1::2], copyfn)
                    nc.scalar.activation(
                        k8v[:, :, 21:37].rearrange("c b (u w) -> c b u w",
                                                   u=4),
                        kfv[:, :, 0:4, 0::2], copyfn)
                    nc.scalar.activation(
                        k8v[:, :, 37:38], kfv[:, :, 4, 0:1], copyfn)
                else:
                    nc.scalar.activation(
                        k8[:], k_feat[mc][:].rearrange("c b f -> c (b f)"),
                        copyfn)
                kr = apool.tile([128, NB, NSLOT, 4], u8, name=f"kr_{mc}")
                if NA < NSLOT:
                    nc.vector.memset(kr[:], 0)
                nc.vector.tensor_copy(
                    kr[:, :, 0:NA, :],
                    k8[:].bitcast(u8)
                        .rearrange("c (b f) -> c b f", b=NB)[:, :, 0:NA]
                        .unsqueeze(3).broadcast_to([128, NB, NA, 4]))
                krep_t.append(kr)

            for _rep in range(repeat):
              # ---- main loop over batch groups -----------------------------
              NGRP = NB // GB
              ys = {}

              def alloc_group(g):
                  """DMA loads + tile allocation for one batch group; conv
                  compute is emitted piecewise via emit_conv_piece."""
                  tiles = {}
                  if SDR:
                      xh = spool.tile([128, GB, NCH, HS * HS], fp8,
                                      name="xh", tag="xh")
                      nc.gpsimd.dma_start(xh[:], xs8h[g][:])
                      xl = spool.tile([128, GB, NCH, HS * HS], fp8l,
                                      name="xl", tag="xl")
                      nc.gpsimd.dma_start(xl[:], xs8l[g][:])
                      tiles["x"] = (xh, xl)
                  else:
                      xb = spool.tile([128, GB, NCH, HS * HS], bf16,
                                      name="xb", tag="xh")
                      nc.gpsimd.dma_start(xb[:], xsbf[g][:])
                      tiles["x"] = (xb,)
                  for mc in range(NCH):
                      s8h = spool.tile([128, GB, HS, HSP], fp8,
                                       name=f"s8h{mc}", tag=f"s8h{mc}")
                      s8l = spool.tile([128, GB, HS, HSP], fp8l,
                                       name=f"s8l{mc}", tag=f"s8l{mc}")
                      sbf = spool.tile([128, GB, HS, HSP], bf16,
                                       name=f"sbf{mc}", tag=f"sbf{mc}")
                      tiles[mc] = (s8h, s8l, sbf)
                  return tiles

              def emit_conv_piece(tiles, bl, mc):
                  """Search conv + epilogues + residual for one (bl, mc).
                  Weight-outer / phase-inner so each conv LDWEIGHTS serves
                  both PSUM phases back-to-back."""
                  s8h, s8l, sbf = tiles[mc]
                  msl = slice(mc * 128, (mc + 1) * 128)
                  for ph, (r0, r1) in enumerate(((0, 16), (16, 31))):
                      ps = ppool.tile([128, r1 - r0, HS], f32,
                                      name="ps_cs2",
                                      tag=("csA" if ph == 0 else "csB"),
                                      bufs=1)
                      fsl = slice(r0 * HS, r1 * HS)
                      if SDR:
                          CDRM = (mybir.MatmulPerfMode.DoubleRowSwInterleave
                                  if DRI else DRM)
                          xh, xl = tiles["x"]
                          nc.tensor.matmul(
                              ps[:], ws8h_t[mc][:],
                              xh[:, bl, :, fsl], start=True,
                              stop=False, perf_mode=CDRM)
                          nc.tensor.matmul(
                              ps[:], ws8h_t[mc][:],
                              xl[:, bl, :, fsl], start=False,
                              stop=False, perf_mode=CDRM)
                          nc.tensor.matmul(
                              ps[:], ws8l_t[mc][:],
                              xh[:, bl, :, fsl], start=False,
                              stop=True, perf_mode=CDRM)
                      else:
                          (xb,) = tiles["x"]
                          for kc in range(NCH):
                              nc.tensor.matmul(
                                  ps[:], ws_t[kc][:, msl],
                                  xb[:, bl, kc, fsl],
                                  start=(kc == 0),
                                  stop=(kc == NCH - 1))
                      nc.scalar.activation(s8h[:, bl, r0:r1, 0:HS], ps[:],
                                           relu, bias=shs_t[mc][:],
                                           scale=scs_t[mc][:])
                      nc.scalar.activation(sbf[:, bl, r0:r1, 0:HS], ps[:],
                                           relu, bias=shs_t[mc][:],
                                           scale=scs_t[mc][:])
                  nc.gpsimd.tensor_tensor(
                      s8l[:, bl, :, 0:HS], sbf[:, bl, :, 0:HS],
                      s8h[:, bl, :, 0:HS],
                      mybir.AluOpType.subtract)

              def emit_group_conv(g):
                  tiles = alloc_group(g)
                  for mc in range(NCH):
                      for bl in range(GB):
                          emit_conv_piece(tiles, bl, mc)
                  return [tiles[mc] for mc in range(NCH)]

              s_feat = emit_group_conv(0)
              tiles_next = None
              piece_idx = 0
              def emit_stacks(b_abs):
                  """Diag stack builds (DVE u32 AND) for both chunks of one
                  batch; hoisted a batch ahead so the PE never waits."""
                  if not NA:
                      return None
                  res = []
                  for mc in range(NCH):
                      st = dpool.tile([128, NSLOT, 32], u32,
                                      name="st", tag=f"st{mc}")
                      nc.vector.tensor_tensor(
                          st[:],
                          krep_t[mc][:, b_abs].bitcast(u32)
                              .broadcast_to([128, NSLOT, 32]),
                          mk_t[:].unsqueeze(1)
                              .broadcast_to([128, NSLOT, 32]),
                          mybir.AluOpType.bitwise_and)
                      res.append(st[:].bitcast(fp8))  # [128, NSLOT, 128]
                  return res

              stacks = emit_stacks(0)
              for g in range(NGRP):
                  s_feat_next = None
                  if g + 1 < NGRP:
                      tiles_next = alloc_group(g + 1)
                      piece_idx = 0
                      s_feat_next = [tiles_next[mc] for mc in range(NCH)]
                  for bl in range(GB):
                      b_abs = g * GB + bl
                      cur_stacks = stacks
                      if b_abs + 1 < NB:
                          stacks = emit_stacks(b_abs + 1)
                      # feat tiles hold both channel chunks: [128, 2, 625]
                      f8h = f8l = None
                      if H1DR:
                          f8h = apool.tile([128, NCH, HO * HO], fp8,
                                           name="f8h", tag="f8h", bufs=2)
                          f8l = apool.tile([128, NCH, HO * HO], fp8l,
                                           name="f8l", tag="f8l", bufs=2)
                      fbf = apool.tile([128, NCH, HO * HO], bf16,
                                       name="fbf", tag="fbf", bufs=2)
                      for mc in range(NCH):
                          s8h, s8l, sbf = s_feat[mc]
                          if NA:
                              st8 = cur_stacks[mc]

                          # bf16 taps NA..48: NPOOL of them on Pool (own
                          # accumulator, merged on Pool), the rest on DVE.
                          # k_feat is row-major in (u, v) regardless of the
                          # TAPS permutation.
                          kcol = lambda uv: k_feat[mc][
                              :, b_abs,
                              TAPS[uv][0] * HK + TAPS[uv][1]:
                              TAPS[uv][0] * HK + TAPS[uv][1] + 1]
                          n_rest = 49 - NA
                          n_pool = min(NPOOL, max(0, n_rest - 1))
                          n_dve = n_rest - n_pool
                          accd = None
                          if n_rest:
                              accd = apool.tile([128, HO, HO], bf16,
                                                name="accd", tag=f"accd{mc}",
                                                bufs=4)
                              for i in range(n_dve):
                                  u, v = TAPS[NA + i]
                                  win = sbf[:, bl, u:u + HO, v:v + HO]
                                  if i == 0:
                                      nc.vector.tensor_scalar_mul(
                                          accd[:], win, kcol(NA + i))
                                  else:
                                      tmp = apool.tile([128, HO, HO], bf16,
                                                       name="tmp", tag="tmp",
                                                       bufs=6)
                                      nc.vector.tensor_scalar_mul(
                                          tmp[:], win, kcol(NA + i))
                                      nc.vector.tensor_tensor(
                                          accd[:], accd[:], tmp[:],
                                          mybir.AluOpType.add)
                              accp = None
                              for i in range(n_pool):
                                  uv = NA + n_dve + i
                                  u, v = TAPS[uv]
                                  win = sbf[:, bl, u:u + HO, v:v + HO]
                                  if i == 0:
                                      accp = apool.tile([128, HO, HO], bf16,
                                                        name="accp",
                                                        tag=f"accp{mc}",
                                                        bufs=4)
                                      nc.gpsimd.tensor_scalar_mul(
                                          accp[:], win, kcol(uv))
                                  else:
                                      tmp = apool.tile([128, HO, HO], bf16,
                                                       name="tmpp", tag="tmpp",
                                                       bufs=6)
                                      nc.gpsimd.tensor_scalar_mul(
                                          tmp[:], win, kcol(uv))
                                      nc.gpsimd.tensor_tensor(
                                          accp[:], accp[:], tmp[:],
                                          mybir.AluOpType.add)
                              if accp is not None:
                                  nc.gpsimd.tensor_tensor(
                                      accd[:], accd[:], accp[:],
                                      mybir.AluOpType.add)

                          # xcorr phases: psA rows 0:20, psB rows 20:25
                          for ph, (r0, r1) in enumerate(((0, 20), (20, 25))):
                              nr = r1 - r0
                              ps = ppool.tile([128, nr * HO], f32,
                                              name=f"ps_xc{ph}", tag="xc",
                                              bufs=3)
                              for t in range(NPAIR):
                                  ua, va = TAPS[2 * t]
                                  if 2 * t + 1 < NA:
                                      ub, vb = TAPS[2 * t + 1]
                                  else:
                                      ub, vb = ua, va  # zero-slot pad
                                  wts = st8[:, 2 * t:2 * t + 2, :]
                                  for j, img in enumerate((s8h, s8l)):
                                      rhs = _pair_ap(AP, img[:], bl, ua, va,
                                                     ub, vb, r0, nr)
                                      nc.tensor.matmul(
                                          ps[:], wts, rhs,
                                          start=(t == 0 and j == 0),
                                          stop=(t == NPAIR - 1 and j == 1
                                                and accd is None),
                                          perf_mode=DRM)
                              if accd is not None:
                                  nc.tensor.matmul(
                                      ps[:], id_t[:], accd[:, r0:r1, :],
                                      start=(NPAIR == 0), stop=True)
                              c0 = r0 * HO
                              c1 = r1 * HO
                              if H1DR:
                                  nc.scalar.activation(f8h[:, mc, c0:c1],
                                                       ps[:], copyfn)
                              nc.scalar.activation(fbf[:, mc, c0:c1], ps[:],
                                                   copyfn)
                          if H1DR:
                              nc.gpsimd.tensor_tensor(
                                  f8l[:, mc], fbf[:, mc], f8h[:, mc],
                                  mybir.AluOpType.subtract)

                          # interleave one conv piece of the next group so
                          # the PE + epilogue engines stay ahead
                          if s_feat_next is not None and piece_idx < NCH * GB:
                              emit_conv_piece(tiles_next, piece_idx // NCH,
                                              piece_idx % NCH)
                              piece_idx += 1

                          if mc < NCH - 1:
                              continue

                          # head conv1 + BN/ReLU
                          for mq in range(NCH):
                              yt = apool.tile([128, HO * HO], bf16,
                                              name=f"y{mq}", tag=f"y{mq}",
                                              bufs=2)
                              qsl = slice(mq * 128, (mq + 1) * 128)
                              for ph, (c0, c1) in enumerate(((0, 500),
                                                            (500, 625))):
                                  ps = ppool.tile([128, c1 - c0], f32,
                                                  name="ps_hd",
                                                  tag=("hdA" if ph == 0
                                                       else "hdB"),
                                                  bufs=(2 if ph == 0 else 1))
                                  if H1DR:
                                      CDRM = (
                                          mybir.MatmulPerfMode
                                          .DoubleRowSwInterleave
                                          if DRI else DRM)
                                      nc.tensor.matmul(
                                          ps[:], wh18h_t[mq][:],
                                          f8h[:, :, c0:c1], start=True,
                                          stop=False, perf_mode=CDRM)
                                      nc.tensor.matmul(
                                          ps[:], wh18h_t[mq][:],
                                          f8l[:, :, c0:c1], start=False,
                                          stop=False, perf_mode=CDRM)
                                      nc.tensor.matmul(
                                          ps[:], wh18l_t[mq][:],
                                          f8h[:, :, c0:c1], start=False,
                                          stop=True, perf_mode=CDRM)
                                  else:
                                      for kc in range(NCH):
                                          nc.tensor.matmul(
                                              ps[:], wh1_t[kc][:, qsl],
                                              fbf[:, kc, c0:c1],
                                              start=(kc == 0),
                                              stop=(kc == NCH - 1))
                                  nc.scalar.activation(yt[:, c0:c1], ps[:],
                                                       relu, bias=shh_t[mq][:],
                                                       scale=sch_t[mq][:])
                              ys[mq] = yt

                          # head conv2 + bias (bf16)
                          ot = apool.tile([COUT, HO * HO], f32,
                                          name="ot", tag="ot", bufs=2)
                          for ph, (c0, c1) in enumerate(((0, 500), (500, 625))):
                              ps = ppool.tile([COUT, c1 - c0], f32,
                                              name="ps_o",
                                              tag=("hdA" if ph == 0 else "hdB"),
                                              bufs=(2 if ph == 0 else 1))
                              for kc in range(NCH):
                                  nc.tensor.matmul(
                                      ps[:],
                                      wh2_t[kc][:],
                                      ys[kc][:, c0:c1],
                                      start=(kc == 0), stop=(kc == NCH - 1),
                                  )
                              nc.scalar.activation(ot[:, c0:c1], ps[:], idfn,
                                                   bias=bh2_t[:], scale=1.0)
                          nc.sync.dma_start(
                              out[b_abs][:].rearrange("o h w -> o (h w)"),
                              ot[:])
                  if s_feat_next is not None:
                      s_feat = s_feat_next

    nc.compile()
    return nc


def _get_nc():
    if "nc" not in _CACHE:
        _CACHE["nc"] = _build_nc()
    return _CACHE["nc"]


def _host_prep(w, dt_hi, dt_lo, scale=1.0):
    ws = np.asarray(w, np.float32) * scale
    hi = ws.astype(dt_hi)
    lo = (ws - hi.astype(np.float32)).astype(dt_lo)
    return hi, lo


def build_in_maps(kernel, search, w_k, g_k, b_k, m_k, v_k, w_s, g_s, b_s,
                  m_s, v_s, w_h1, g_h, b_h, m_h, v_h, w_h2, bias_h2):
    def fold(g, b, m, v, wscale=1.0):
        sc = (np.asarray(g) / np.sqrt(np.asarray(v) + EPS) / wscale)
        sh = (np.asarray(b) - np.asarray(m) * sc * wscale)
        return (sc.astype(np.float32).reshape(-1, 1),
                sh.astype(np.float32).reshape(-1, 1))

    kernel, search, w_k, w_s, w_h1, w_h2, bias_h2 = [
        np.asarray(a) for a in
        (kernel, search, w_k, w_s, w_h1, w_h2, bias_h2)]

    global DRI
    sck, shk = fold(g_k, b_k, m_k, v_k)
    scs, shs = fold(g_s, b_s, m_s, v_s, WSCALE if SDR else 1.0)
    sch, shh = fold(g_h, b_h, m_h, v_h, WSCALE if H1DR else 1.0)

    # fp8 hi/lo conv weights, pre-scaled x16, laid out [kc, 128, HID]
    # (transposed: contraction dim on rows)
    wsT = np.ascontiguousarray(w_s.T).astype(np.float32)      # [CIN, HID]
    wh1T = np.ascontiguousarray(w_h1.T).astype(np.float32)
    ws8h, ws8l = _host_prep(wsT, E4M3, E4M3, WSCALE)
    wh18h, wh18l = _host_prep(wh1T, E4M3, E4M3, WSCALE)

    def _interleave(w):
        # [256, 256] -> per 128-out-chunk DoubleRowSwInterleave layout:
        # stored[:, 2k] = A[:, M-1-k], stored[:, 2k+1] = B[:, M-1-k]
        # where A/B are the two k-tiles; returns same [NCH, 128, HID] shape
        # consumed by the per-chunk tile loads.
        r = np.empty((NCH, 128, HID), w.dtype)
        wv = w.reshape(NCH, 128, HID)
        for mq in range(NCH):
            A = wv[0, :, mq * 128:(mq + 1) * 128]
            B = wv[1, :, mq * 128:(mq + 1) * 128]
            st = np.empty((128, 256), w.dtype)
            st[:, 0::2] = A[:, ::-1]
            st[:, 1::2] = B[:, ::-1]
            # store back into the [t, c, m] layout the DMA rearrange expects:
            # tile [c, t, m] = st[c, t + 2*?]... tile slot t, col m reads
            # stored flat index 2*m + t? No: tile AP [c, 2, 128] reads
            # flat [c, 256]; keep flat layout in slot-major order.
            r[0, :, mq * 128:(mq + 1) * 128] = st[:, 0:128]
            r[1, :, mq * 128:(mq + 1) * 128] = st[:, 128:256]
        return r

    if DRI:
        ws8h, ws8l = _interleave(ws8h.reshape(NCH, 128, HID)), _interleave(
            ws8l.reshape(NCH, 128, HID))
        wh18h, wh18l = _interleave(
            wh18h.reshape(NCH, 128, HID)), _interleave(
            wh18l.reshape(NCH, 128, HID))

    mask32 = np.zeros((128, 32), np.uint32)
    for c in range(128):
        mask32[c, c // 4] = np.uint32(0xFF) << np.uint32(8 * (c % 4))

    common = {
        "wkT": np.ascontiguousarray(w_k.T).astype(BF16),
        "wsT": wsT.astype(BF16),
        "wh1T": wh1T.astype(BF16),
        "ws8h": ws8h.reshape(NCH, 128, HID),
        "ws8l": ws8l.reshape(NCH, 128, HID),
        "wh18h": wh18h.reshape(NCH, 128, HID),
        "wh18l": wh18l.reshape(NCH, 128, HID),
        "wh2T": np.ascontiguousarray(w_h2.T).astype(BF16),
        "sck": sck, "shk": shk, "scs": scs, "shs": shs,
        "sch": sch, "shh": shh,
        "bh2": bias_h2.astype(np.float32).reshape(-1, 1),
        "ident": np.eye(128, dtype=BF16),
        "mask32": mask32,
    }
    xk_all = kernel.astype(BF16)
    xs = search.reshape(B, CIN, HS * HS).astype(np.float32)
    xs8h_all = xs.astype(E4M3)
    xs8l_all = (xs - xs8h_all.astype(np.float32)).astype(E5M2)

    def _grp(a):
        # [NB, 256, F] -> [NGRP, 128, GB, NCH, F] (tile layout per group)
        ngrp = NB // GB
        return np.ascontiguousarray(
            a.reshape(ngrp, GB, NCH, 128, HS * HS).transpose(0, 3, 1, 2, 4))

    in_maps = []
    for i in range(NCORES):
        bs = slice(i * NB, (i + 1) * NB)
        m = dict(common)
        m["xk"] = np.ascontiguousarray(xk_all[bs])
        m["xs8h"] = _grp(xs8h_all[bs])
        m["xs8l"] = _grp(xs8l_all[bs])
        m["xsbf"] = _grp(xs[bs].astype(BF16))
        in_maps.append(m)
    return in_maps


def kernel(**inputs):
    from concourse.bass_utils import run_bass_kernel_spmd

    in_maps = build_in_maps(**{k: np.asarray(v) for k, v in inputs.items()})
    nc = _get_nc()
    res = run_bass_kernel_spmd(nc, in_maps, core_ids=list(range(NCORES)))
    return np.concatenate([res.results[i]["out"] for i in range(NCORES)],
                          axis=0)

